# revision 31
# baseline (speedup 1.0000x reference)
"""Trainium2 Bass kernel for the NeuromorphicCore problem.

Computes, distributed over 8 NeuronCores (column-parallel over D=8192):
    drive = x @ W                     (once; loop-invariant)
    100-step LIF scan:  pot += drive; spike = pot >= 0.5; pot *= (1-spike)
    synaptic_activity = mean(|W|)

Sharding: core m owns columns [m*1024, (m+1)*1024) of W, of drive, and of the
membrane state. No cross-device traffic; host gathers/concatenates.

Per-core pipeline:
  - stream W slice (32MB) as 64 [128, 1024] tiles (512KB DMAs) — the memory
    roofline (~90us at ~358GB/s per core)
  - PE: fused f32r matmuls with stationary [x_chunk | ones] -> PSUM rows
    {drive, column_sums}; column sums give sum(W) = sum(|W|) since W >= 0
    (uniform fill; kernel() falls back to a host computation otherwise)
  - drive is loop-invariant, so the LIF scan has a closed form per element
    (drive v, threshold th, zero initial state):
        K = ceil(th / v)               (spike period, steps)
        m_t = (t+1) mod K
        spike_t = (m_t == 0);  pot_t = v * m_t
    computed chunk-wise ([128, 100] tiles, time on the free axis, per-
    partition scalars v and K), avoiding the 100-step serial dependency.
  - PE transposes flip [128, 100] chunks into time-major [100, 1024] SBUF
    accumulators; outputs leave via two fully-contiguous DMAs.
"""

import numpy as np

import concourse.bacc as bacc
import concourse.mybir as mybir
import concourse.tile as tile
from concourse.bass_utils import run_bass_kernel_spmd

D = 8192
NCORES = 8
DLOC = D // NCORES            # 1024 columns per core
T = 100                       # time steps
THRESH = 0.5
KT = D // 128                 # 64 contraction tiles
NCHUNK = DLOC // 128          # 8 scan chunks of 128 elements
FP32 = mybir.dt.float32
FP32R = mybir.dt.float32r
I32 = mybir.dt.int32
ALU = mybir.AluOpType


def _body(tc, xs_d, w_d, spk_d, pot_d, ws_d, repeat=1, sub=1, alt=False):
    nc = tc.nc
    import contextlib

    with contextlib.ExitStack() as ctx:
        wp = ctx.enter_context(tc.tile_pool(name="wp", bufs=4))
        small = ctx.enter_context(tc.tile_pool(name="small", bufs=1))
        ch = ctx.enter_context(tc.tile_pool(name="ch", bufs=3))
        psp = ctx.enter_context(tc.tile_pool(name="psp", bufs=1, space="PSUM"))
        ptp = ctx.enter_context(tc.tile_pool(name="ptp", bufs=2, space="PSUM"))

        if repeat > 1:
            with tc.For_i(0, repeat, 1):
                _emit(nc, small, wp, ch, psp, ptp, xs_d, w_d, spk_d, pot_d,
                      ws_d, sub, alt)
        else:
            _emit(nc, small, wp, ch, psp, ptp, xs_d, w_d, spk_d, pot_d,
                  ws_d, sub, alt)


def _emit(nc, small, wp, ch, psp, ptp, xs_d, w_d, spk_d, pot_d, ws_d,
          stream_sub=1, stream_alt=False):
        # stationary operand [x | ones], host-prepared: col j < KT is x chunk
        # j (x[j*128+p]), col KT+j is all-ones. lhsT for chunk j = cols
        # {j, KT+j} (stride KT), giving PSUM rows {drive, colsum}.
        xs = small.tile([128, 2 * KT], FP32R)
        nc.sync.dma_start(out=xs[:, :], in_=xs_d[:, :])

        # (t+1) along the free axis, f32, shared by all chunks
        t1i = small.tile([128, T], I32)
        nc.gpsimd.iota(t1i[:, :], pattern=[[1, T]], base=1, channel_multiplier=0)
        t1 = small.tile([128, T], FP32)
        nc.vector.tensor_scalar(out=t1[:, :], in0=t1i[:, :], scalar1=0.0,
                                scalar2=None, op0=ALU.add)

        # 128x128 identity for the PE transposes (id2 slice serves [2, 128])
        id128 = small.tile([128, 128], FP32)
        nc.vector.memset(id128[:, :], 1.0)
        nc.gpsimd.affine_select(id128[:, :], id128[:, :], pattern=[[-1, 128]],
                                compare_op=ALU.is_equal, fill=0.0,
                                base=0, channel_multiplier=1)

        ps = psp.tile([2, DLOC], FP32)
        xs_pairs = xs[:, :].rearrange("p (two j) -> p j two", two=2)

        # each tile holds SUB contraction chunks (SUB*128 W rows)
        SUB = stream_sub
        for j in range(KT // SUB):
            wt = wp.tile([128, SUB * DLOC], FP32R)
            eng = nc.scalar if (stream_alt and j % 2) else nc.sync
            eng.dma_start(
                out=wt[:, :].rearrange("p (sub col) -> p sub col", sub=SUB),
                in_=w_d[j * SUB * 128:(j + 1) * SUB * 128, :].rearrange(
                    "(sub p) col -> p sub col", sub=SUB),
            )
            for sub in range(SUB):
                jj = j * SUB + sub
                for nb in range(2):
                    nc.tensor.matmul(
                        ps[0:2, nb * 512:(nb + 1) * 512],
                        lhsT=xs_pairs[:, jj, :],
                        rhs=wt[:, sub * DLOC + nb * 512:sub * DLOC + (nb + 1) * 512],
                        start=(jj == 0),
                        stop=(jj == KT - 1),
                    )

        # PSUM rows -> SBUF [2, DLOC] -> PE transposes of [2, 128] blocks
        # -> tp16[p, 2c:2c+2] = (drive, colsum)[c*128 + p] (chunk-major).
        dcs = small.tile([2, DLOC], FP32)
        nc.scalar.copy(out=dcs[:, :], in_=ps[0:2, :])
        tp16 = psp.tile([128, 2 * NCHUNK], FP32)
        id2 = id128[0:2, 0:2]
        for c in range(NCHUNK):
            nc.tensor.transpose(tp16[:, 2 * c:2 * c + 2],
                                dcs[:, c * 128:(c + 1) * 128], id2)
        tp_pairs = tp16[:, :].rearrange("p (c two) -> p two c", two=2)
        dv = small.tile([128, NCHUNK], FP32)
        nc.vector.tensor_copy(dv[:, :], tp_pairs[:, 0, :])
        cs = small.tile([128, NCHUNK], FP32)
        nc.vector.tensor_copy(cs[:, :], tp_pairs[:, 1, :])
        nc.sync.dma_start(out=ws_d[:, :], in_=cs[:, :])

        # K = clamp(ceil(THRESH/v), 1, 127) per element. The f32->i32
        # convert truncates in CoreSim but rounds-to-nearest on HW, so use
        # it only to land in {floor, floor+1} and correct with a compare:
        # ceil(x) = k0 + (k0 < x). The HW reciprocal is f32-accurate.
        rv = small.tile([128, NCHUNK], FP32)
        nc.vector.reciprocal(rv[:, :], dv[:, :])
        kf = small.tile([128, NCHUNK], FP32)
        nc.vector.tensor_scalar(out=kf[:, :], in0=rv[:, :], scalar1=THRESH,
                                scalar2=126.5, op0=ALU.mult, op1=ALU.min)
        k0i = small.tile([128, NCHUNK], I32)
        nc.vector.tensor_copy(k0i[:, :], kf[:, :])
        k0 = small.tile([128, NCHUNK], FP32)
        nc.vector.tensor_scalar(out=k0[:, :], in0=k0i[:, :], scalar1=0.0,
                                scalar2=None, op0=ALU.add)
        cc = small.tile([128, NCHUNK], FP32)
        nc.vector.tensor_tensor(out=cc[:, :], in0=k0[:, :], in1=kf[:, :],
                                op=ALU.is_lt)
        ks = small.tile([128, NCHUNK], FP32)
        nc.vector.tensor_tensor(out=ks[:, :], in0=k0[:, :], in1=cc[:, :],
                                op=ALU.add)
        kt = small.tile([128, NCHUNK], FP32)
        nc.vector.tensor_scalar(out=kt[:, :], in0=ks[:, :], scalar1=1.0,
                                scalar2=None, op0=ALU.max)
        rk = small.tile([128, NCHUNK], FP32)
        nc.vector.reciprocal(rk[:, :], kt[:, :])

        # time-major output accumulators
        st_s = small.tile([T, DLOC], FP32)
        st_p = small.tile([T, DLOC], FP32)

        for c in range(NCHUNK):
            # m = (t+1) mod K without convert-rounding assumptions:
            # q = convert(T1/K) lands in {floor, floor+1}; m' = T1 - K*q
            # is then in [-K, K); add K where negative.
            qf = ch.tile([128, T], FP32)
            nc.vector.tensor_scalar(out=qf[:, :], in0=t1[:, :],
                                    scalar1=rk[:, c:c + 1], scalar2=None,
                                    op0=ALU.mult)
            qi = ch.tile([128, T], I32)
            nc.vector.tensor_copy(qi[:, :], qf[:, :])
            qk = ch.tile([128, T], FP32)
            nc.vector.tensor_scalar(out=qk[:, :], in0=qi[:, :],
                                    scalar1=kt[:, c:c + 1], scalar2=None,
                                    op0=ALU.mult)
            mn = ch.tile([128, T], FP32)
            nc.vector.tensor_tensor(out=mn[:, :], in0=t1[:, :], in1=qk[:, :],
                                    op=ALU.subtract)
            fx = ch.tile([128, T], FP32)
            nc.vector.tensor_scalar(out=fx[:, :], in0=mn[:, :], scalar1=0.0,
                                    scalar2=kt[:, c:c + 1],
                                    op0=ALU.is_lt, op1=ALU.mult)
            mc = ch.tile([128, T], FP32)
            nc.vector.tensor_tensor(out=mc[:, :], in0=mn[:, :], in1=fx[:, :],
                                    op=ALU.add)
            sc = ch.tile([128, T], FP32)
            nc.vector.tensor_scalar(out=sc[:, :], in0=mc[:, :], scalar1=0.0,
                                    scalar2=None, op0=ALU.is_equal)
            pc = ch.tile([128, T], FP32)
            nc.vector.tensor_scalar(out=pc[:, :], in0=mc[:, :],
                                    scalar1=dv[:, c:c + 1], scalar2=None,
                                    op0=ALU.mult)
            tps = ptp.tile([T, 128], FP32)
            nc.tensor.transpose(tps[:, :], sc[:, :], id128[:, :])
            tpp = ptp.tile([T, 128], FP32)
            nc.tensor.transpose(tpp[:, :], pc[:, :], id128[:, :])
            nc.scalar.copy(out=st_s[:, c * 128:(c + 1) * 128], in_=tps[:, :])
            nc.scalar.copy(out=st_p[:, c * 128:(c + 1) * 128], in_=tpp[:, :])

        # two HWDGE rings (sync + scalar) so the output stores overlap
        nc.sync.dma_start(out=spk_d[:, :], in_=st_s[:, :])
        nc.scalar.dma_start(out=pot_d[:, :], in_=st_p[:, :])


def build_nc(repeat=1, sub=2, alt=False):
    nc = bacc.Bacc("TRN2", target_bir_lowering=False, debug=False,
                   num_devices=NCORES)
    xs_d = nc.dram_tensor("xs", [128, 2 * KT], FP32R, kind="ExternalInput").ap()
    w_d = nc.dram_tensor("w", [D, DLOC], FP32R, kind="ExternalInput").ap()
    spk_d = nc.dram_tensor("spikes", [T, DLOC], FP32, kind="ExternalOutput").ap()
    pot_d = nc.dram_tensor("pots", [T, DLOC], FP32, kind="ExternalOutput").ap()
    ws_d = nc.dram_tensor("wsum", [128, NCHUNK], FP32, kind="ExternalOutput").ap()
    with tile.TileContext(nc) as tc:
        _body(tc, xs_d, w_d, spk_d, pot_d, ws_d, repeat=repeat, sub=sub, alt=alt)
    nc.compile()
    return nc


_NC_CACHE = None


def _get_nc():
    global _NC_CACHE
    if _NC_CACHE is None:
        _NC_CACHE = build_nc()
    return _NC_CACHE


def make_in_maps(x, synaptic_weights, membrane_potential):
    x = np.ascontiguousarray(np.asarray(x), dtype=np.float32)
    w = np.asarray(synaptic_weights)
    # [x | ones] stationary operand, shared by all cores
    xs = np.concatenate(
        [x.reshape(KT, 128).T, np.ones((128, KT), np.float32)], axis=1
    )
    xs = np.ascontiguousarray(xs, dtype=np.float32)
    return [
        {
            "xs": xs,
            "w": np.ascontiguousarray(w[:, m * DLOC:(m + 1) * DLOC], dtype=np.float32),
        }
        for m in range(NCORES)
    ]


def gather(results):
    spikes = np.concatenate([results[m]["spikes"] for m in range(NCORES)], axis=1)
    pots = np.concatenate([results[m]["pots"] for m in range(NCORES)], axis=1)
    total = sum(r["wsum"].astype(np.float64).sum() for r in results)
    act = np.float32(total / (D * D))
    return spikes, pots, act


def _host_reference(x, w, p0):
    # numpy fallback for input regimes outside the device fast path
    drive = (x @ w).astype(np.float32)
    pot = p0.astype(np.float32).copy()
    spikes = np.empty((T, D), np.float32)
    pots = np.empty((T, D), np.float32)
    for t in range(T):
        pot = pot + drive
        s = (pot >= THRESH).astype(np.float32)
        pot = pot * (1.0 - s)
        spikes[t] = s
        pots[t] = pot
    act = np.float32(np.abs(w).mean(dtype=np.float64))
    return spikes, pots, act


def kernel(x, synaptic_weights, membrane_potential):
    x = np.ascontiguousarray(np.asarray(x), dtype=np.float32)
    w = np.asarray(synaptic_weights)
    p0 = np.asarray(membrane_potential)
    if p0.any() or float(w.min()) < 0.0:
        # device path assumes zero initial membrane state and W >= 0
        # (both guaranteed by the problem's input spec)
        return _host_reference(x, w.astype(np.float32), p0)
    nc = _get_nc()
    in_maps = make_in_maps(x, w, p0)
    res = run_bass_kernel_spmd(nc, in_maps, list(range(NCORES)))
    return gather(res.results)
